# revision 14
# baseline (speedup 1.0000x reference)
"""MoE layer (top-2 routing) on 8 Trainium2 NeuronCores.

Strategy (expert-parallel, per the sharding hint):
  - Router (logits -> softmax -> top-2 -> combine weights, aux loss) is
    computed on host with the exact same eager jax ops as the reference,
    so routing decisions match the reference bitwise.
  - Tokens are dispatched (host-side all-to-all) to 8 expert shards: core e
    receives the tokens whose top-2 set contains expert e, padded to a
    common capacity C.
  - Each core runs the expert FFN  y = relu(x @ W1[e] + b1[e]) @ W2[e] + b2[e]
    as a Bass/Tile kernel: tokens live in the matmul free dimension
    (activations stay transposed, [D, C]), so no on-device transposes are
    needed anywhere.  Matmuls run in float32r (TF32-like, ~1.5e-4 rel err,
    4x the fp32 matmul rate), accumulation in fp32 PSUM.
  - Host combines: out[t] = g_a * y_ea[t] + g_b * y_eb[t]  (expert-index
    order, matching the reference's accumulation order).
"""

import numpy as np

B, S, D, F, E = 8, 2048, 512, 2048, 8
P = 128
NCH = 512                 # tokens per chunk == PSUM bank free dim (fp32)
KD, MF = D // P, F // P   # 4, 16  (mm1: K-tiles over D, M-tiles over F)
KF, MD = F // P, D // P   # 16, 4  (mm2: K-tiles over F, M-tiles over D)

_PROGRAM_CACHE = {}


def chunk_sizes(C):
    """Split C into token chunks: full 512s plus a tail chunk.  Every chunk
    must be >=256 (f32r matmul runs at 1 cycle/row only when the moving dim
    is >=256), so a short remainder is folded into the last two chunks."""
    assert C % 8 == 0 and C >= 512
    nfull, rem = divmod(C, NCH)
    if rem == 0:
        return [NCH] * nfull
    if rem >= 256:
        return [NCH] * nfull + [rem]
    c1 = ((NCH + rem) // 2 + 7) // 8 * 8
    return [NCH] * (nfull - 1) + [c1, NCH + rem - c1]


def capacity_for(max_count):
    """Smallest supported capacity >= max_count (8-aligned, >=512)."""
    return max(NCH, (int(max_count) + 7) // 8 * 8)


def build_ffn_program(C, reps=1, dt_name="float32r"):
    """Per-core expert-FFN program: yT[D,C] = FFN(xT[D,C]) with weights resident."""
    import concourse.bacc as bacc
    import concourse.tile as tile
    from concourse import mybir

    DT = getattr(mybir.dt, dt_name)
    f32 = mybir.dt.float32
    AF = mybir.ActivationFunctionType
    chunks = chunk_sizes(C)

    nc = bacc.Bacc(None, target_bir_lowering=False, debug=False)
    xT = nc.dram_tensor("xT", [KD, P, C], DT, kind="ExternalInput")
    w1 = nc.dram_tensor("w1", [KD, P, F], DT, kind="ExternalInput")
    w2 = nc.dram_tensor("w2", [KF, P, D], DT, kind="ExternalInput")
    b1 = nc.dram_tensor("b1", [P, MF], f32, kind="ExternalInput")
    b2 = nc.dram_tensor("b2", [P, MD], f32, kind="ExternalInput")
    yT = nc.dram_tensor("yT", [MD, P, C], f32, kind="ExternalOutput")

    with tile.TileContext(nc) as tc:
        with (
            tc.tile_pool(name="wpool", bufs=1) as wpool,
            tc.tile_pool(name="xpool", bufs=3) as xpool,
            tc.tile_pool(name="hpool", bufs=2) as hpool,
            tc.tile_pool(name="ypool", bufs=3) as ypool,
            tc.tile_pool(name="ps1", bufs=8, space="PSUM") as ps1,
        ):
            w1t = wpool.tile([P, KD, F], DT, tag="w1")
            w2t = wpool.tile([P, KF, D], DT, tag="w2")
            b1t = wpool.tile([P, MF], f32, tag="b1")
            b2t = wpool.tile([P, MD], f32, tag="b2")
            # Startup order matters: the HWDGE queue drains in FIFO order.
            # Chunk 0's x goes first, then b1, then w1 in m-major blocks of
            # 512 columns (the first matmul group only needs block 0, so PE
            # starts after ~2MB instead of ~5MB), then w2 (only needed once
            # chunk 0 reaches mm2).
            def load_w1():
                nc.sync.dma_start(b1t[:], b1[:])
                for mb in range(MF * P // 512):
                    for k in range(KD):
                        bs = slice(mb * 512, (mb + 1) * 512)
                        nc.sync.dma_start(w1t[:, k, bs], w1[k][:, bs])

            def load_w2():
                for k in range(KF):
                    nc.sync.dma_start(w2t[:, k, :], w2[k])
                nc.sync.dma_start(b2t[:], b2[:])

            def body(first=False):
                off = 0
                for ci, cw in enumerate(chunks):
                    sl = slice(off, off + cw)
                    off += cw
                    xt = xpool.tile([P, KD, NCH], DT, tag="x")
                    for k in range(KD):
                        nc.sync.dma_start(xt[:, k, :cw], xT[k, :, sl])
                    if first and ci == 0:
                        load_w1()
                        load_w2()
                    ht = hpool.tile([P, KF, NCH], DT, tag="h")
                    for m in range(MF):
                        ps = ps1.tile([P, NCH], f32, tag="ps1")
                        for k in range(KD):
                            nc.tensor.matmul(
                                ps[:, :cw],
                                lhsT=w1t[:, k, m * P:(m + 1) * P],
                                rhs=xt[:, k, :cw],
                                start=(k == 0),
                                stop=(k == KD - 1),
                            )
                        nc.scalar.activation(ht[:, m, :cw], ps[:, :cw], AF.Relu,
                                             bias=b1t[:, m:m + 1])
                    yt = ypool.tile([P, MD, NCH], f32, tag="y")
                    for m in range(MD):
                        ps = ps1.tile([P, NCH], f32, tag="ps1")
                        for k in range(KF):
                            nc.tensor.matmul(
                                ps[:, :cw],
                                lhsT=w2t[:, k, m * P:(m + 1) * P],
                                rhs=ht[:, k, :cw],
                                start=(k == 0),
                                stop=(k == KF - 1),
                            )
                        nc.scalar.activation(yt[:, m, :cw], ps[:, :cw],
                                             AF.Identity, bias=b2t[:, m:m + 1])
                        nc.sync.dma_start(yT[m, :, sl], yt[:, m, :cw])

            if reps == 1:
                body(first=True)
            else:
                body(first=True)
                with tc.For_i(0, reps - 1):
                    body()

    nc.finalize()
    return nc


def _get_program(C, reps=1, dt_name="float32r"):
    key = (C, reps, dt_name)
    if key not in _PROGRAM_CACHE:
        _PROGRAM_CACHE[key] = build_ffn_program(C, reps, dt_name)
    return _PROGRAM_CACHE[key]


def route_host(x, Wr, br):
    """Router computed with the reference's exact eager jax ops (bitwise match)."""
    import jax
    import jax.numpy as jnp

    logits = jnp.einsum('bsd,de->bse', jnp.asarray(x), jnp.asarray(Wr)) \
        + jnp.asarray(br)
    gate = jax.nn.softmax(logits, axis=-1)
    top2_val, top2_idx = jax.lax.top_k(gate, 2)
    expert_prob = gate.mean(axis=(0, 1))
    aux_loss = jnp.sum(expert_prob * jnp.log(expert_prob + 1e-9))
    return (np.asarray(top2_val), np.asarray(top2_idx),
            np.float32(np.asarray(aux_loss)))


def make_dispatch(t2i):
    """Token lists / slots per expert from the [T,2] top-2 index array."""
    T = t2i.shape[0]
    e1, e2 = t2i[:, 0], t2i[:, 1]
    toks = [np.nonzero((e1 == e) | (e2 == e))[0] for e in range(E)]
    counts = np.array([len(t) for t in toks])
    slot = np.zeros((T, 2), np.int64)
    for e in range(E):
        p_of = np.empty(T, np.int64)
        p_of[toks[e]] = np.arange(len(toks[e]))
        for r in range(2):
            m = t2i[:, r] == e
            slot[m, r] = p_of[m]
    return toks, counts, slot


def _ensure_axon_hooks_stub():
    """bass_utils' trace path imports antenv.axon_hooks, which is absent in
    this axon client build; give it a no-op hook so a BASS_TRACE env var
    can't crash the run (trace degrades gracefully to no-trace)."""
    try:
        import antenv.axon_hooks  # noqa: F401
    except ImportError:
        import sys
        import types
        m = types.ModuleType("antenv.axon_hooks")
        m.get_axon_ntff_profile_hook = lambda: None
        sys.modules["antenv.axon_hooks"] = m


def kernel(x, Wr, br, W1, b1, W2, b2, _reps=1, _dt_name="float32r",
           _return_results=False):
    _ensure_axon_hooks_stub()
    from concourse.bass_utils import run_bass_kernel_spmd

    x = np.asarray(x, np.float32)
    Wr = np.asarray(Wr, np.float32)
    br = np.asarray(br, np.float32)
    W1 = np.asarray(W1, np.float32)
    b1 = np.asarray(b1, np.float32)
    W2 = np.asarray(W2, np.float32)
    b2 = np.asarray(b2, np.float32)

    T = B * S
    x_flat = x.reshape(T, D)

    top2_val, top2_idx, aux_loss = route_host(x, Wr, br)
    t2i = top2_idx.reshape(T, 2)
    t2v = top2_val.reshape(T, 2)

    toks, counts, slot = make_dispatch(t2i)
    C = capacity_for(counts.max())

    in_maps = []
    for e in range(E):
        xe = np.zeros((C, D), np.float32)
        xe[:counts[e]] = x_flat[toks[e]]
        in_maps.append({
            "xT": np.ascontiguousarray(xe.T).reshape(KD, P, C),
            "w1": np.ascontiguousarray(W1[e]).reshape(KD, P, F),
            "w2": np.ascontiguousarray(W2[e]).reshape(KF, P, D),
            "b1": np.ascontiguousarray(b1[e].reshape(MF, P).T),
            "b2": np.ascontiguousarray(b2[e].reshape(MD, P).T),
        })

    nc = _get_program(C, _reps, _dt_name)
    res = run_bass_kernel_spmd(nc, in_maps, list(range(E)), trace=False)

    # y_stack[e, c, :] = expert e's output for its c-th assigned token
    y_stack = np.stack([res.results[e]["yT"].reshape(D, C).T for e in range(E)])

    # Combine in expert-index order (matches the reference's e-loop order)
    e1, e2 = t2i[:, 0], t2i[:, 1]
    r_first = np.where(e1 < e2, 0, 1)
    ar = np.arange(T)
    ga = t2v[ar, r_first]
    gb = t2v[ar, 1 - r_first]
    ea = t2i[ar, r_first]
    eb = t2i[ar, 1 - r_first]
    sa = slot[ar, r_first]
    sb = slot[ar, 1 - r_first]
    out_flat = ga[:, None] * y_stack[ea, sa] + gb[:, None] * y_stack[eb, sb]
    out = out_flat.reshape(B, S, D).astype(np.float32)

    if _return_results:
        return out, aux_loss, res
    return out, aux_loss


# revision 16
# speedup vs baseline: 1.0123x; 1.0123x over previous
"""MoE layer (top-2 routing) on 8 Trainium2 NeuronCores.

Strategy (expert-parallel, per the sharding hint):
  - Router (logits -> softmax -> top-2 -> combine weights, aux loss) is
    computed on host with the exact same eager jax ops as the reference,
    so routing decisions match the reference bitwise.
  - Tokens are dispatched (host-side all-to-all) to 8 expert shards: core e
    receives the tokens whose top-2 set contains expert e, padded to a
    common capacity C.
  - Each core runs the expert FFN  y = relu(x @ W1[e] + b1[e]) @ W2[e] + b2[e]
    as a Bass/Tile kernel: tokens live in the matmul free dimension
    (activations stay transposed, [D, C]), so no on-device transposes are
    needed anywhere.  Matmuls run in float32r (TF32-like, ~1.5e-4 rel err,
    4x the fp32 matmul rate), accumulation in fp32 PSUM.
  - Host combines: out[t] = g_a * y_ea[t] + g_b * y_eb[t]  (expert-index
    order, matching the reference's accumulation order).
"""

import numpy as np

B, S, D, F, E = 8, 2048, 512, 2048, 8
P = 128
NCH = 512                 # tokens per chunk == PSUM bank free dim (fp32)
KD, MF = D // P, F // P   # 4, 16  (mm1: K-tiles over D, M-tiles over F)
KF, MD = F // P, D // P   # 16, 4  (mm2: K-tiles over F, M-tiles over D)

_PROGRAM_CACHE = {}


def chunk_sizes(C):
    """Split C into token chunks: full 512s plus a tail chunk.  Every chunk
    must be >=256 (f32r matmul runs at 1 cycle/row only when the moving dim
    is >=256), so a short remainder is folded into the last two chunks."""
    assert C % 8 == 0 and C >= 512
    nfull, rem = divmod(C, NCH)
    if rem == 0:
        return [NCH] * nfull
    if rem >= 256:
        return [NCH] * nfull + [rem]
    c1 = ((NCH + rem) // 2 + 7) // 8 * 8
    return [NCH] * (nfull - 1) + [c1, NCH + rem - c1]


def capacity_for(max_count):
    """Smallest supported capacity >= max_count (8-aligned, >=512)."""
    return max(NCH, (int(max_count) + 7) // 8 * 8)


def build_ffn_program(C, reps=1, dt_name="float32r"):
    """Per-core expert-FFN program: yT[D,C] = FFN(xT[D,C]) with weights resident."""
    import concourse.bacc as bacc
    import concourse.tile as tile
    from concourse import mybir

    DT = getattr(mybir.dt, dt_name)
    f32 = mybir.dt.float32
    AF = mybir.ActivationFunctionType
    chunks = chunk_sizes(C)

    nc = bacc.Bacc(None, target_bir_lowering=False, debug=False)
    xT = nc.dram_tensor("xT", [KD, P, C], DT, kind="ExternalInput")
    w1 = nc.dram_tensor("w1", [KD, P, F], DT, kind="ExternalInput")
    w2 = nc.dram_tensor("w2", [KF, P, D], DT, kind="ExternalInput")
    b1 = nc.dram_tensor("b1", [P, MF], f32, kind="ExternalInput")
    b2 = nc.dram_tensor("b2", [P, MD], f32, kind="ExternalInput")
    yT = nc.dram_tensor("yT", [MD, P, C], f32, kind="ExternalOutput")

    with tile.TileContext(nc) as tc:
        with (
            tc.tile_pool(name="wpool", bufs=1) as wpool,
            tc.tile_pool(name="xpool", bufs=3) as xpool,
            tc.tile_pool(name="hpool", bufs=2) as hpool,
            tc.tile_pool(name="ypool", bufs=3) as ypool,
            tc.tile_pool(name="ps1", bufs=4, space="PSUM") as ps1,
            tc.tile_pool(name="ps2", bufs=4, space="PSUM") as ps2,
        ):
            w1t = wpool.tile([P, KD, F], DT, tag="w1")
            w2t = wpool.tile([P, KF, D], DT, tag="w2")
            b1t = wpool.tile([P, MF], f32, tag="b1")
            b2t = wpool.tile([P, MD], f32, tag="b2")
            # Startup order matters: the HWDGE queue drains in FIFO order.
            # Chunk 0's x goes first, then b1, then w1 in m-major blocks of
            # 512 columns (the first matmul group only needs block 0, so PE
            # starts after ~2MB instead of ~5MB), then w2 (only needed once
            # chunk 0 reaches mm2).
            def load_w1_rest():
                for mb in range(1, MF * P // 512):
                    for k in range(KD):
                        bs = slice(mb * 512, (mb + 1) * 512)
                        nc.sync.dma_start(w1t[:, k, bs], w1[k][:, bs])

            def load_w2():
                for k in range(KF):
                    nc.sync.dma_start(w2t[:, k, :], w2[k])
                nc.sync.dma_start(b2t[:], b2[:])

            def body(first=False):
                off = 0
                for ci, cw in enumerate(chunks):
                    sl = slice(off, off + cw)
                    off += cw
                    xt = xpool.tile([P, KD, NCH], DT, tag="x")
                    if first and ci == 0:
                        # Interleave chunk-0 x with w1 block 0 k-by-k: the
                        # k=0 matmul of (m=0) fires after ~0.5MB of DMA.
                        nc.sync.dma_start(b1t[:], b1[:])
                        for k in range(KD):
                            nc.sync.dma_start(xt[:, k, :cw], xT[k, :, sl])
                            nc.sync.dma_start(w1t[:, k, 0:512], w1[k][:, 0:512])
                        load_w1_rest()
                        load_w2()
                    else:
                        for k in range(KD):
                            nc.sync.dma_start(xt[:, k, :cw], xT[k, :, sl])
                    ht = hpool.tile([P, KF, NCH], DT, tag="h")
                    for m in range(MF):
                        ps = ps1.tile([P, NCH], f32, tag="ps1")
                        for k in range(KD):
                            nc.tensor.matmul(
                                ps[:, :cw],
                                lhsT=w1t[:, k, m * P:(m + 1) * P],
                                rhs=xt[:, k, :cw],
                                start=(k == 0),
                                stop=(k == KD - 1),
                            )
                        nc.scalar.activation(ht[:, m, :cw], ps[:, :cw], AF.Relu,
                                             bias=b1t[:, m:m + 1])
                    yt = ypool.tile([P, MD, NCH], f32, tag="y")
                    for m in range(MD):
                        ps = ps2.tile([P, NCH], f32, tag="ps2")
                        for k in range(KF):
                            nc.tensor.matmul(
                                ps[:, :cw],
                                lhsT=w2t[:, k, m * P:(m + 1) * P],
                                rhs=ht[:, k, :cw],
                                start=(k == 0),
                                stop=(k == KF - 1),
                            )
                        nc.scalar.activation(yt[:, m, :cw], ps[:, :cw],
                                             AF.Identity, bias=b2t[:, m:m + 1])
                        nc.sync.dma_start(yT[m, :, sl], yt[:, m, :cw])

            if reps == 1:
                body(first=True)
            else:
                body(first=True)
                with tc.For_i(0, reps - 1):
                    body()

    nc.finalize()
    return nc


def _get_program(C, reps=1, dt_name="float32r"):
    key = (C, reps, dt_name)
    if key not in _PROGRAM_CACHE:
        _PROGRAM_CACHE[key] = build_ffn_program(C, reps, dt_name)
    return _PROGRAM_CACHE[key]


def route_host(x, Wr, br):
    """Router computed with the reference's exact eager jax ops (bitwise match)."""
    import jax
    import jax.numpy as jnp

    logits = jnp.einsum('bsd,de->bse', jnp.asarray(x), jnp.asarray(Wr)) \
        + jnp.asarray(br)
    gate = jax.nn.softmax(logits, axis=-1)
    top2_val, top2_idx = jax.lax.top_k(gate, 2)
    expert_prob = gate.mean(axis=(0, 1))
    aux_loss = jnp.sum(expert_prob * jnp.log(expert_prob + 1e-9))
    return (np.asarray(top2_val), np.asarray(top2_idx),
            np.float32(np.asarray(aux_loss)))


def make_dispatch(t2i):
    """Token lists / slots per expert from the [T,2] top-2 index array."""
    T = t2i.shape[0]
    e1, e2 = t2i[:, 0], t2i[:, 1]
    toks = [np.nonzero((e1 == e) | (e2 == e))[0] for e in range(E)]
    counts = np.array([len(t) for t in toks])
    slot = np.zeros((T, 2), np.int64)
    for e in range(E):
        p_of = np.empty(T, np.int64)
        p_of[toks[e]] = np.arange(len(toks[e]))
        for r in range(2):
            m = t2i[:, r] == e
            slot[m, r] = p_of[m]
    return toks, counts, slot


def _ensure_axon_hooks_stub():
    """bass_utils' trace path imports antenv.axon_hooks, which is absent in
    this axon client build; give it a no-op hook so a BASS_TRACE env var
    can't crash the run (trace degrades gracefully to no-trace)."""
    try:
        import antenv.axon_hooks  # noqa: F401
    except ImportError:
        import sys
        import types
        m = types.ModuleType("antenv.axon_hooks")
        m.get_axon_ntff_profile_hook = lambda: None
        sys.modules["antenv.axon_hooks"] = m


def kernel(x, Wr, br, W1, b1, W2, b2, _reps=1, _dt_name="float32r",
           _return_results=False):
    _ensure_axon_hooks_stub()
    from concourse.bass_utils import run_bass_kernel_spmd

    x = np.asarray(x, np.float32)
    Wr = np.asarray(Wr, np.float32)
    br = np.asarray(br, np.float32)
    W1 = np.asarray(W1, np.float32)
    b1 = np.asarray(b1, np.float32)
    W2 = np.asarray(W2, np.float32)
    b2 = np.asarray(b2, np.float32)

    T = B * S
    x_flat = x.reshape(T, D)

    top2_val, top2_idx, aux_loss = route_host(x, Wr, br)
    t2i = top2_idx.reshape(T, 2)
    t2v = top2_val.reshape(T, 2)

    toks, counts, slot = make_dispatch(t2i)
    C = capacity_for(counts.max())

    in_maps = []
    for e in range(E):
        xe = np.zeros((C, D), np.float32)
        xe[:counts[e]] = x_flat[toks[e]]
        in_maps.append({
            "xT": np.ascontiguousarray(xe.T).reshape(KD, P, C),
            "w1": np.ascontiguousarray(W1[e]).reshape(KD, P, F),
            "w2": np.ascontiguousarray(W2[e]).reshape(KF, P, D),
            "b1": np.ascontiguousarray(b1[e].reshape(MF, P).T),
            "b2": np.ascontiguousarray(b2[e].reshape(MD, P).T),
        })

    nc = _get_program(C, _reps, _dt_name)
    res = run_bass_kernel_spmd(nc, in_maps, list(range(E)), trace=False)

    # y_stack[e, c, :] = expert e's output for its c-th assigned token
    y_stack = np.stack([res.results[e]["yT"].reshape(D, C).T for e in range(E)])

    # Combine in expert-index order (matches the reference's e-loop order)
    e1, e2 = t2i[:, 0], t2i[:, 1]
    r_first = np.where(e1 < e2, 0, 1)
    ar = np.arange(T)
    ga = t2v[ar, r_first]
    gb = t2v[ar, 1 - r_first]
    ea = t2i[ar, r_first]
    eb = t2i[ar, 1 - r_first]
    sa = slot[ar, r_first]
    sb = slot[ar, 1 - r_first]
    out_flat = ga[:, None] * y_stack[ea, sa] + gb[:, None] * y_stack[eb, sb]
    out = out_flat.reshape(B, S, D).astype(np.float32)

    if _return_results:
        return out, aux_loss, res
    return out, aux_loss


# revision 21
# speedup vs baseline: 1.0906x; 1.0774x over previous
"""MoE layer (top-2 routing) on 8 Trainium2 NeuronCores.

Strategy (expert-parallel, per the sharding hint):
  - Router (logits -> softmax -> top-2 -> combine weights, aux loss) is
    computed on host with the exact same eager jax ops as the reference,
    so routing decisions match the reference bitwise.
  - Tokens are dispatched (host-side all-to-all) to 8 expert shards: core e
    receives the tokens whose top-2 set contains expert e, padded to a
    common capacity C.
  - Each core runs the expert FFN  y = relu(x @ W1[e] + b1[e]) @ W2[e] + b2[e]
    as a Bass/Tile kernel: tokens live in the matmul free dimension
    (activations stay transposed, [D, C]), so no on-device transposes are
    needed anywhere.  Matmuls run in float32r (TF32-like, ~1.5e-4 rel err,
    4x the fp32 matmul rate), accumulation in fp32 PSUM.
  - Host combines: out[t] = g_a * y_ea[t] + g_b * y_eb[t]  (expert-index
    order, matching the reference's accumulation order).
"""

import numpy as np

B, S, D, F, E = 8, 2048, 512, 2048, 8
P = 128
NCH = 512                 # tokens per chunk == PSUM bank free dim (fp32)
KD, MF = D // P, F // P   # 4, 16  (mm1: K-tiles over D, M-tiles over F)
KF, MD = F // P, D // P   # 16, 4  (mm2: K-tiles over F, M-tiles over D)

_PROGRAM_CACHE = {}


def chunk_sizes(C):
    """Split C into token chunks: full 512s plus a tail chunk.  Every chunk
    must be >=256 (f32r matmul runs at 1 cycle/row only when the moving dim
    is >=256), so a short remainder is folded into the last two chunks."""
    assert C % 8 == 0 and C >= 512
    nfull, rem = divmod(C, NCH)
    if rem == 0:
        return [NCH] * nfull
    if rem >= 256:
        return [NCH] * nfull + [rem]
    c1 = ((NCH + rem) // 2 + 7) // 8 * 8
    return [NCH] * (nfull - 1) + [c1, NCH + rem - c1]


def capacity_for(max_count):
    """Smallest supported capacity >= max_count (8-aligned, >=512)."""
    return max(NCH, (int(max_count) + 7) // 8 * 8)


def build_ffn_program(C, reps=1, dt_name="float32r"):
    """Per-core expert-FFN program: yT[D,C] = FFN(xT[D,C]) with weights resident."""
    import concourse.bacc as bacc
    import concourse.tile as tile
    from concourse import mybir

    DT = getattr(mybir.dt, dt_name)
    f32 = mybir.dt.float32
    AF = mybir.ActivationFunctionType
    chunks = chunk_sizes(C)

    nc = bacc.Bacc(None, target_bir_lowering=False, debug=False)
    xT = nc.dram_tensor("xT", [KD, P, C], DT, kind="ExternalInput")
    w1 = nc.dram_tensor("w1", [KD, P, F], DT, kind="ExternalInput")
    w2 = nc.dram_tensor("w2", [KF, P, D], DT, kind="ExternalInput")
    b1 = nc.dram_tensor("b1", [P, MF], f32, kind="ExternalInput")
    b2 = nc.dram_tensor("b2", [P, MD], f32, kind="ExternalInput")
    yT = nc.dram_tensor("yT", [MD, P, C], f32, kind="ExternalOutput")

    with tile.TileContext(nc) as tc:
        with (
            tc.tile_pool(name="wpool", bufs=1) as wpool,
            tc.tile_pool(name="xpool", bufs=3) as xpool,
            tc.tile_pool(name="hpool", bufs=2) as hpool,
            tc.tile_pool(name="ypool", bufs=3) as ypool,
            tc.tile_pool(name="ps1", bufs=4, space="PSUM") as ps1,
            tc.tile_pool(name="ps2", bufs=4, space="PSUM") as ps2,
        ):
            w1t = wpool.tile([P, KD, F], DT, tag="w1")
            w2t = wpool.tile([P, KF, D], DT, tag="w2")
            b1t = wpool.tile([P, MF], f32, tag="b1")
            b2t = wpool.tile([P, MD], f32, tag="b2")
            # Startup order matters: the HWDGE queue drains in FIFO order.
            # Chunk 0's x goes first, then b1, then w1 in m-major blocks of
            # 512 columns (the first matmul group only needs block 0, so PE
            # starts after ~2MB instead of ~5MB), then w2 (only needed once
            # chunk 0 reaches mm2).
            def load_w1_rest():
                for mb in range(1, MF * P // 512):
                    for k in range(KD):
                        bs = slice(mb * 512, (mb + 1) * 512)
                        nc.sync.dma_start(w1t[:, k, bs], w1[k][:, bs])

            def load_w2():
                for k in range(KF):
                    nc.sync.dma_start(w2t[:, k, :], w2[k])
                nc.sync.dma_start(b2t[:], b2[:])

            def body(first=False):
                off = 0
                for ci, cw in enumerate(chunks):
                    sl = slice(off, off + cw)
                    off += cw
                    xt = xpool.tile([P, KD, NCH], DT, tag="x")
                    if first and ci == 0:
                        # Interleave chunk-0 x with w1 block 0 k-by-k: the
                        # k=0 matmul of (m=0) fires after ~0.5MB of DMA.
                        nc.sync.dma_start(b1t[:], b1[:])
                        for k in range(KD):
                            nc.sync.dma_start(xt[:, k, :cw], xT[k, :, sl])
                            nc.sync.dma_start(w1t[:, k, 0:512], w1[k][:, 0:512])
                        load_w1_rest()
                        load_w2()
                    else:
                        for k in range(KD):
                            nc.sync.dma_start(xt[:, k, :cw], xT[k, :, sl])
                    ht = hpool.tile([P, KF, NCH], DT, tag="h")
                    for m in range(MF):
                        ps = ps1.tile([P, NCH], f32, tag="ps1")
                        for k in range(KD):
                            nc.tensor.matmul(
                                ps[:, :cw],
                                lhsT=w1t[:, k, m * P:(m + 1) * P],
                                rhs=xt[:, k, :cw],
                                start=(k == 0),
                                stop=(k == KD - 1),
                            )
                        nc.scalar.activation(ht[:, m, :cw], ps[:, :cw], AF.Relu,
                                             bias=b1t[:, m:m + 1])
                    yt = ypool.tile([P, MD, NCH], f32, tag="y")
                    for m in range(MD):
                        ps = ps2.tile([P, NCH], f32, tag="ps2")
                        for k in range(KF):
                            nc.tensor.matmul(
                                ps[:, :cw],
                                lhsT=w2t[:, k, m * P:(m + 1) * P],
                                rhs=ht[:, k, :cw],
                                start=(k == 0),
                                stop=(k == KF - 1),
                            )
                        nc.scalar.activation(yt[:, m, :cw], ps[:, :cw],
                                             AF.Identity, bias=b2t[:, m:m + 1])
                        nc.sync.dma_start(yT[m, :, sl], yt[:, m, :cw])

            if reps == 1:
                body(first=True)
            else:
                body(first=True)
                with tc.For_i(0, reps - 1):
                    body()

    nc.finalize()
    return nc


def _get_program(C, reps=1, dt_name="float32r"):
    key = (C, reps, dt_name)
    if key not in _PROGRAM_CACHE:
        _PROGRAM_CACHE[key] = build_ffn_program(C, reps, dt_name)
    return _PROGRAM_CACHE[key]


FH = F // 2        # F-half width per split worker (1024)
KH = FH // P       # 8: mm1 m-tiles / mm2 k-tiles per worker


def build_split_program(CA, CB, reps=1, dt_name="float32r"):
    """Per-core program running TWO half-F expert workers sequentially.

    Worker = (expert, F-half): mm1 computes h_half = relu(x@W1[:,half]+b1[half])
    ([FH, C] in SBUF), mm2 computes the partial y = h_half @ W2[half,:] for the
    full D columns.  The two halves of one expert live on different cores; the
    host sums them (and adds b2).  Slot A capacity CA holds the big experts,
    slot B capacity CB the small ones, cutting per-core rows vs one uniform
    capacity."""
    import concourse.bacc as bacc
    import concourse.tile as tile
    from concourse import mybir

    DT = getattr(mybir.dt, dt_name)
    f32 = mybir.dt.float32
    AF = mybir.ActivationFunctionType

    nc = bacc.Bacc(None, target_bir_lowering=False, debug=False)
    io = {}
    for tag, C in (("a", CA), ("b", CB)):
        io["x" + tag] = nc.dram_tensor("x" + tag, [KD, P, C], DT,
                                       kind="ExternalInput")
        io["w1" + tag] = nc.dram_tensor("w1" + tag, [KD, P, FH], DT,
                                        kind="ExternalInput")
        io["w2" + tag] = nc.dram_tensor("w2" + tag, [KH, P, D], DT,
                                        kind="ExternalInput")
        io["b1" + tag] = nc.dram_tensor("b1" + tag, [P, KH], f32,
                                        kind="ExternalInput")
        io["y" + tag] = nc.dram_tensor("y" + tag, [MD, P, C], f32,
                                       kind="ExternalOutput")

    with tile.TileContext(nc) as tc:
        with (
            tc.tile_pool(name="wpool", bufs=1) as wpool,
            tc.tile_pool(name="xpool", bufs=3) as xpool,
            tc.tile_pool(name="hpool", bufs=2) as hpool,
            tc.tile_pool(name="ypool", bufs=3) as ypool,
            tc.tile_pool(name="ps1", bufs=4, space="PSUM") as ps1,
            tc.tile_pool(name="ps2", bufs=4, space="PSUM") as ps2,
        ):
            sb = {}
            for tag in ("a", "b"):
                sb["w1" + tag] = wpool.tile([P, KD, FH], DT, tag="w1" + tag,
                                            name="w1t" + tag)
                sb["w2" + tag] = wpool.tile([P, KH, D], DT, tag="w2" + tag,
                                            name="w2t" + tag)
                sb["b1" + tag] = wpool.tile([P, KH], f32, tag="b1" + tag,
                                            name="b1t" + tag)

            def w1_block(tag, mb):
                bs = slice(mb * 512, (mb + 1) * 512)
                for k in range(KD):
                    nc.sync.dma_start(sb["w1" + tag][:, k, bs],
                                      io["w1" + tag][k][:, bs])

            def w2_load(tag):
                for k in range(KH):
                    nc.sync.dma_start(sb["w2" + tag][:, k, :],
                                      io["w2" + tag][k])

            def phase(tag, C, first=False):
                xT, yT = io["x" + tag], io["y" + tag]
                w1t, w2t, b1t = sb["w1" + tag], sb["w2" + tag], sb["b1" + tag]
                off = 0
                for ci, cw in enumerate(chunk_sizes(C)):
                    sl = slice(off, off + cw)
                    off += cw
                    xt = xpool.tile([P, KD, NCH], DT, tag="x")
                    other = "b" if tag == "a" else "a"
                    if first and ci == 0:
                        # First chunk of the whole kernel: interleave x with
                        # w1 block 0 so the first matmul fires ~0.5MB in,
                        # then stream this worker's remaining weights in
                        # consumption order (w1 blk1, w2, w1 blk2-3).
                        nc.sync.dma_start(b1t[:], io["b1" + tag][:])
                        for k in range(KD):
                            nc.sync.dma_start(xt[:, k, :cw], xT[k, :, sl])
                            nc.sync.dma_start(w1t[:, k, 0:512],
                                              io["w1" + tag][k][:, 0:512])
                        w1_block(tag, 1)
                        w2_load(tag)
                        for mb in range(2, FH // 512):
                            w1_block(tag, mb)
                    else:
                        for k in range(KD):
                            nc.sync.dma_start(xt[:, k, :cw], xT[k, :, sl])
                        if first and ci == 1:
                            # Other worker's weights stream behind chunk 1's
                            # x, well before phase B needs them.
                            nc.sync.dma_start(sb["b1" + other][:],
                                              io["b1" + other][:])
                            for mb in range(FH // 512):
                                w1_block(other, mb)
                        elif first and ci == 2:
                            w2_load(other)
                    ht = hpool.tile([P, KH, NCH], DT, tag="h")
                    for m in range(KH):
                        ps = ps1.tile([P, NCH], f32, tag="ps1")
                        for k in range(KD):
                            nc.tensor.matmul(
                                ps[:, :cw],
                                lhsT=w1t[:, k, m * P:(m + 1) * P],
                                rhs=xt[:, k, :cw],
                                start=(k == 0),
                                stop=(k == KD - 1),
                            )
                        nc.scalar.activation(ht[:, m, :cw], ps[:, :cw],
                                             AF.Relu, bias=b1t[:, m:m + 1])
                    yt = ypool.tile([P, MD, NCH], f32, tag="y")
                    for m in range(MD):
                        ps = ps2.tile([P, NCH], f32, tag="ps2")
                        for k in range(KH):
                            nc.tensor.matmul(
                                ps[:, :cw],
                                lhsT=w2t[:, k, m * P:(m + 1) * P],
                                rhs=ht[:, k, :cw],
                                start=(k == 0),
                                stop=(k == KH - 1),
                            )
                        nc.scalar.copy(yt[:, m, :cw], ps[:, :cw])
                        nc.sync.dma_start(yT[m, :, sl], yt[:, m, :cw])

            def body(first=False):
                phase("a", CA, first=first)
                phase("b", CB)

            if reps == 1:
                body(first=True)
            else:
                body(first=True)
                with tc.For_i(0, reps - 1):
                    body()

    nc.finalize()
    return nc


def plan_split(counts):
    """Rank experts by load: slot A (cap = biggest count) gets the 4 biggest
    experts' halves, slot B (cap = 5th biggest) the rest.  Both halves of an
    expert share a slot type; worker (rank r, half h) -> core 2*(r%4)+h."""
    order = np.argsort(-np.asarray(counts), kind="stable")
    CA = capacity_for(counts[order[0]])
    CB = capacity_for(counts[order[4]])
    coreof = {}
    for r, e in enumerate(order):
        slot = 0 if r < 4 else 1
        for h in (0, 1):
            coreof[(int(e), h)] = (2 * (r % 4) + h, slot)
    return CA, CB, coreof


def route_host(x, Wr, br):
    """Router computed with the reference's exact eager jax ops (bitwise match)."""
    import jax
    import jax.numpy as jnp

    logits = jnp.einsum('bsd,de->bse', jnp.asarray(x), jnp.asarray(Wr)) \
        + jnp.asarray(br)
    gate = jax.nn.softmax(logits, axis=-1)
    top2_val, top2_idx = jax.lax.top_k(gate, 2)
    expert_prob = gate.mean(axis=(0, 1))
    aux_loss = jnp.sum(expert_prob * jnp.log(expert_prob + 1e-9))
    return (np.asarray(top2_val), np.asarray(top2_idx),
            np.float32(np.asarray(aux_loss)))


def make_dispatch(t2i):
    """Token lists / slots per expert from the [T,2] top-2 index array."""
    T = t2i.shape[0]
    e1, e2 = t2i[:, 0], t2i[:, 1]
    toks = [np.nonzero((e1 == e) | (e2 == e))[0] for e in range(E)]
    counts = np.array([len(t) for t in toks])
    slot = np.zeros((T, 2), np.int64)
    for e in range(E):
        p_of = np.empty(T, np.int64)
        p_of[toks[e]] = np.arange(len(toks[e]))
        for r in range(2):
            m = t2i[:, r] == e
            slot[m, r] = p_of[m]
    return toks, counts, slot


def _ensure_axon_hooks_stub():
    """bass_utils' trace path imports antenv.axon_hooks, which is absent in
    this axon client build; give it a no-op hook so a BASS_TRACE env var
    can't crash the run (trace degrades gracefully to no-trace)."""
    try:
        import antenv.axon_hooks  # noqa: F401
    except ImportError:
        import sys
        import types
        m = types.ModuleType("antenv.axon_hooks")
        m.get_axon_ntff_profile_hook = lambda: None
        sys.modules["antenv.axon_hooks"] = m


def kernel(x, Wr, br, W1, b1, W2, b2, _reps=1, _dt_name="float32r",
           _return_results=False):
    _ensure_axon_hooks_stub()
    from concourse.bass_utils import run_bass_kernel_spmd

    x = np.asarray(x, np.float32)
    Wr = np.asarray(Wr, np.float32)
    br = np.asarray(br, np.float32)
    W1 = np.asarray(W1, np.float32)
    b1 = np.asarray(b1, np.float32)
    W2 = np.asarray(W2, np.float32)
    b2 = np.asarray(b2, np.float32)

    T = B * S
    x_flat = x.reshape(T, D)

    top2_val, top2_idx, aux_loss = route_host(x, Wr, br)
    t2i = top2_idx.reshape(T, 2)
    t2v = top2_val.reshape(T, 2)

    toks, counts, slot = make_dispatch(t2i)
    CA, CB, coreof = plan_split(counts)
    cap = {0: CA, 1: CB}

    in_maps = [{} for _ in range(E)]
    for e in range(E):
        core0, s = coreof[(e, 0)]
        C_e = cap[s]
        xe = np.zeros((C_e, D), np.float32)
        xe[:counts[e]] = x_flat[toks[e]]
        xTe = np.ascontiguousarray(xe.T).reshape(KD, P, C_e)
        for h in (0, 1):
            core, s = coreof[(e, h)]
            tag = "a" if s == 0 else "b"
            fh = slice(h * FH, (h + 1) * FH)
            in_maps[core]["x" + tag] = xTe
            in_maps[core]["w1" + tag] = np.ascontiguousarray(
                W1[e][:, fh]).reshape(KD, P, FH)
            in_maps[core]["w2" + tag] = np.ascontiguousarray(
                W2[e][fh, :]).reshape(KH, P, D)
            in_maps[core]["b1" + tag] = np.ascontiguousarray(
                b1[e][fh].reshape(KH, P).T)

    key = ("split", CA, CB, _reps, _dt_name)
    if key not in _PROGRAM_CACHE:
        _PROGRAM_CACHE[key] = build_split_program(CA, CB, _reps, _dt_name)
    nc = _PROGRAM_CACHE[key]
    res = run_bass_kernel_spmd(nc, in_maps, list(range(E)), trace=False)

    # y_stack[e, c, :] = expert e's output for its c-th assigned token:
    # sum of the two half-F partial products, plus b2.
    Cmax = max(CA, CB)
    y_stack = np.zeros((E, Cmax, D), np.float32)
    for e in range(E):
        parts = []
        for h in (0, 1):
            core, s = coreof[(e, h)]
            tag = "a" if s == 0 else "b"
            C_e = cap[s]
            parts.append(res.results[core]["y" + tag].reshape(D, C_e).T)
        y_stack[e, :C_e] = parts[0] + parts[1] + b2[e]

    # Combine in expert-index order (matches the reference's e-loop order)
    e1, e2 = t2i[:, 0], t2i[:, 1]
    r_first = np.where(e1 < e2, 0, 1)
    ar = np.arange(T)
    ga = t2v[ar, r_first]
    gb = t2v[ar, 1 - r_first]
    ea = t2i[ar, r_first]
    eb = t2i[ar, 1 - r_first]
    sa = slot[ar, r_first]
    sb = slot[ar, 1 - r_first]
    out_flat = ga[:, None] * y_stack[ea, sa] + gb[:, None] * y_stack[eb, sb]
    out = out_flat.reshape(B, S, D).astype(np.float32)

    if _return_results:
        return out, aux_loss, res
    return out, aux_loss
